# revision 1
# baseline (speedup 1.0000x reference)
"""Causal self-attention (B=2, T=2048, C=1024, H=16, RoPE) on 8 TRN2 cores.

Sharding: data-parallel over B (2 groups of 4 cores) x tensor-parallel over
heads (4 heads per core). Each core computes q/k/v projections for its heads,
RoPE, causal attention, and its partial output projection; the host sums the
4 partial projections per batch and adds bp.

Layout choices (per core):
  - xT [C, T] resident in SBUF (contraction dim C on partitions).
  - q, k produced TRANSPOSED: qT/kT [256=4heads*64, T] via lhsT=W, rhs=xT.
    Head-dim pairs are pre-permuted (evens|odds) in the weights so RoPE
    needs no strided access; the pair-swap is a constant permutation
    matmul (J), combine on VectorE with f32 cos/sin.
  - v produced NON-transposed: [T, 256] via lhsT=xT, rhs=WvT.
  - scores computed transposed: ST[tk, tq] = k_rot @ q_rot^T per head, so
    softmax-exp is elementwise (ScalarE, scale=1/8 folded in), the causal
    mask is a fixed 128x128 triangle on diagonal blocks, fully-masked
    blocks are skipped, and P@V needs no transposes.
  - softmax denominators: VectorE accumulates exp-blocks, one ones-vector
    matmul reduces over partitions, reciprocal, then a constant broadcast
    matmul (EA) expands denominators back over partitions for the scale.
"""

import math

import numpy as np
import ml_dtypes

import concourse.bass as bass
import concourse.bacc as bacc
import concourse.mybir as mybir
from concourse.tile import TileContext
from concourse.bass_utils import run_bass_kernel_spmd

BF16 = mybir.dt.bfloat16
F32 = mybir.dt.float32
NPBF16 = ml_dtypes.bfloat16

N_CORES = 8
P = 128

_UNIFIED_ACT_SET = "natural_log_exp_and_others"


def _patch_act_tables():
    import concourse.hw_specs as _hw
    import concourse.bacc as _bacc
    if getattr(_bacc, "_act_tables_patched", False):
        return
    _orig = _hw.get_activation_tables

    def _gat(arch):
        tabs = _orig(arch)
        if _UNIFIED_ACT_SET in tabs:
            keep = tabs[_UNIFIED_ACT_SET]
            drop = {
                mybir.ActivationFunctionType.Exp,
                mybir.ActivationFunctionType.Copy,
            } & keep
            for name, fns in tabs.items():
                if name != _UNIFIED_ACT_SET:
                    for f in drop:
                        fns.discard(f)
        return tabs

    _bacc.get_activation_tables = _gat
    _bacc._act_tables_patched = True


def build_attention_kernel(nc, T=2048, C=1024, n_heads=4, hd=64):
    """Emit the per-core kernel. Returns nothing; tensors are declared on nc."""
    _patch_act_tables()
    HD = n_heads * hd            # 256: local head dims
    KC = C // P                  # 8: contraction chunks for projections
    NJC = HD // P                # 2: partition tiles of qT/kT (head pairs)
    TQB = 512                    # tq block for scores/PV
    NQB = T // TQB               # 4
    NKC = T // P                 # 16: tk chunks
    NTT = T // P                 # 16: t tiles for v
    scale = 1.0 / math.sqrt(hd)

    # ---- DRAM I/O ----
    xT = nc.declare_dram_parameter("xT", [C, T], BF16, isOutput=False)
    wqT = nc.declare_dram_parameter("wqT", [C, HD], BF16, isOutput=False)
    wkT = nc.declare_dram_parameter("wkT", [C, HD], BF16, isOutput=False)
    wvT = nc.declare_dram_parameter("wvT", [C, HD], BF16, isOutput=False)
    wpT = nc.declare_dram_parameter("wpT", [HD, C], BF16, isOutput=False)
    cosq = nc.declare_dram_parameter("cosq", [P, T], F32, isOutput=False)
    sinsq = nc.declare_dram_parameter("sinsq", [P, T], F32, isOutput=False)
    jmat = nc.declare_dram_parameter("jmat", [P, P], BF16, isOutput=False)
    tri = nc.declare_dram_parameter("tri", [P, P], BF16, isOutput=False)
    ea = nc.declare_dram_parameter("ea", [P, P], BF16, isOutput=False)
    onesc = nc.declare_dram_parameter("onesc", [P, 1], BF16, isOutput=False)
    bqT = nc.declare_dram_parameter("bqT", [P, NJC], F32, isOutput=False)
    bkT = nc.declare_dram_parameter("bkT", [P, NJC], F32, isOutput=False)
    bvb = nc.declare_dram_parameter("bvb", [P, HD], F32, isOutput=False)
    z = nc.declare_dram_parameter("z", [T, C], F32, isOutput=True)

    with TileContext(nc) as tc:
        import contextlib

        with contextlib.ExitStack() as ctx:
            # ---- persistent SBUF pools ----
            pc = ctx.enter_context(tc.tile_pool(name="const", bufs=1))
            px = ctx.enter_context(tc.tile_pool(name="x", bufs=1))
            pw = ctx.enter_context(tc.tile_pool(name="w", bufs=1))
            pqk = ctx.enter_context(tc.tile_pool(name="qk", bufs=1))
            pv = ctx.enter_context(tc.tile_pool(name="v", bufs=1))
            py = ctx.enter_context(tc.tile_pool(name="y", bufs=1))
            # transient pools
            pf32 = ctx.enter_context(tc.tile_pool(name="f32tmp", bufs=2))
            prt = ctx.enter_context(tc.tile_pool(name="ropetmp", bufs=2))
            pexp = ctx.enter_context(tc.tile_pool(name="exp", bufs=6))
            pacc = ctx.enter_context(tc.tile_pool(name="acc", bufs=3))
            prcp = ctx.enter_context(tc.tile_pool(name="rcp", bufs=3))
            # PSUM pools
            pmm = ctx.enter_context(
                tc.tile_pool(name="mm", bufs=2, space="PSUM"))
            pyt = ctx.enter_context(
                tc.tile_pool(name="yt", bufs=2, space="PSUM"))

            # ---- constant + weight loads ----
            t_j = pc.tile([P, P], BF16, tag="j")
            nc.gpsimd.dma_start(t_j[:], jmat[:])
            t_tri = pc.tile([P, P], BF16, tag="tri")
            nc.gpsimd.dma_start(t_tri[:], tri[:])
            t_ea = pc.tile([P, P], BF16, tag="ea")
            nc.gpsimd.dma_start(t_ea[:], ea[:])
            t_ones = pc.tile([P, 1], BF16, tag="ones")
            nc.gpsimd.dma_start(t_ones[:], onesc[:])
            t_bq = pc.tile([P, NJC], F32, tag="bq")
            nc.gpsimd.dma_start(t_bq[:], bqT[:])
            t_bk = pc.tile([P, NJC], F32, tag="bk")
            nc.gpsimd.dma_start(t_bk[:], bkT[:])
            t_bv = pc.tile([P, HD], F32, tag="bv")
            nc.gpsimd.dma_start(t_bv[:], bvb[:])
            t_cos = pc.tile([P, T], F32, tag="cos")
            nc.gpsimd.dma_start(t_cos[:], cosq[:])
            t_sin = pc.tile([P, T], F32, tag="sin")
            nc.gpsimd.dma_start(t_sin[:], sinsq[:])
            # persistent staging tile for softmax denominators (rows 0/64
            # carry data; the rest must be finite zeros for the EA matmul)
            t_scp = pc.tile([P, 512], BF16, tag="scp")
            nc.vector.memset(t_scp[:], 0.0)

            t_wq = []
            t_wk = []
            t_wv = []
            for k in range(KC):
                wq_t = pw.tile([P, HD], BF16, tag=f"wq{k}")
                nc.gpsimd.dma_start(wq_t[:], wqT[k * P:(k + 1) * P, :])
                t_wq.append(wq_t)
                wk_t = pw.tile([P, HD], BF16, tag=f"wk{k}")
                nc.gpsimd.dma_start(wk_t[:], wkT[k * P:(k + 1) * P, :])
                t_wk.append(wk_t)
                wv_t = pw.tile([P, HD], BF16, tag=f"wv{k}")
                nc.gpsimd.dma_start(wv_t[:], wvT[k * P:(k + 1) * P, :])
                t_wv.append(wv_t)
            t_wp = []
            for jc in range(NJC):
                wp_t = pw.tile([P, C], BF16, tag=f"wp{jc}")
                nc.gpsimd.dma_start(wp_t[:], wpT[jc * P:(jc + 1) * P, :])
                t_wp.append(wp_t)

            t_x = []
            for k in range(KC):
                x_t = px.tile([P, T], BF16, tag=f"x{k}")
                nc.gpsimd.dma_start(x_t[:], xT[k * P:(k + 1) * P, :])
                t_x.append(x_t)

            # ---- v projection: v[t, dv] in 16 tiles [128, HD] ----
            t_v = []
            for tt in range(NTT):
                vps = pmm.tile([P, TQB], F32, tag="mm")
                for k in range(KC):
                    nc.tensor.matmul(
                        vps[:, 0:HD],
                        lhsT=t_x[k][:, tt * P:(tt + 1) * P],
                        rhs=t_wv[k][:],
                        start=(k == 0),
                        stop=(k == KC - 1),
                    )
                # v layout [128, 4*65]: head i at cols [i*65, i*65+64),
                # a ones column at i*65+64 (PV with it computes the softmax
                # denominator for free as an extra output row)
                v_t = pv.tile([P, n_heads * (hd + 1)], BF16, tag=f"v{tt}")
                v3 = v_t[:].rearrange("p (h c) -> p h c", h=n_heads)
                nc.vector.tensor_add(
                    v3[:, :, 0:hd],
                    vps[:, 0:HD].rearrange("p (h c) -> p h c", h=n_heads),
                    t_bv[:].rearrange("p (h c) -> p h c", h=n_heads),
                )
                nc.gpsimd.memset(v3[:, :, hd:hd + 1], 1.0)
                t_v.append(v_t)

            # ---- q/k projections (transposed) + RoPE ----
            # qrot/krot: NJC tiles [128, T] bf16
            t_qrot = [pqk.tile([P, T], BF16, tag=f"qr{jc}", name=f"qrot{jc}")
                      for jc in range(NJC)]
            t_krot = [pqk.tile([P, T], BF16, tag=f"kr{jc}", name=f"krot{jc}")
                      for jc in range(NJC)]

            for jc in range(NJC):
                for (wchunks, bias, dst) in (
                    (t_wq, t_bq, t_qrot[jc]),
                    (t_wk, t_bk, t_krot[jc]),
                ):
                    raw = pf32.tile([P, T], BF16, tag="qkraw")
                    for tb in range(T // TQB):
                        qps = pmm.tile([P, TQB], F32, tag="mm")
                        for k in range(KC):
                            nc.tensor.matmul(
                                qps[:],
                                lhsT=wchunks[k][:, jc * P:(jc + 1) * P],
                                rhs=t_x[k][:, tb * TQB:(tb + 1) * TQB],
                                start=(k == 0),
                                stop=(k == KC - 1),
                            )
                        # evacuate + bias (bias cols are per-partition scalars)
                        nc.vector.tensor_scalar_add(
                            raw[:, tb * TQB:(tb + 1) * TQB],
                            qps[:],
                            bias[:, jc:jc + 1],
                        )
                    # RoPE: rot = cos*raw + sins*(J@raw)
                    RW = min(1024, T)
                    for half in range(T // RW):
                        sl = slice(half * RW, (half + 1) * RW)
                        jps = pmm.tile([P, RW], F32, tag="mm")
                        for qtr in range(RW // TQB):
                            nc.tensor.matmul(
                                jps[:, qtr * TQB:(qtr + 1) * TQB],
                                lhsT=t_j[:],
                                rhs=raw[:, sl][:, qtr * TQB:(qtr + 1) * TQB],
                            )
                        tmp1 = prt.tile([P, RW], F32, tag="rope1")
                        nc.vector.tensor_mul(tmp1[:], raw[:, sl], t_cos[:, sl])
                        tmp2 = prt.tile([P, RW], F32, tag="rope2")
                        nc.vector.tensor_mul(tmp2[:], jps[:], t_sin[:, sl])
                        nc.vector.tensor_add(dst[:, sl], tmp1[:], tmp2[:])

            # ---- y_norm accumulators ----
            t_yn = [py.tile([P, T], BF16, tag=f"yn{jc}", name=f"yn{jc}")
                    for jc in range(NJC)]

            # ---- attention (qb outer so proj/z-DMA interleave) ----
            for qb in range(NQB):
                for hp in range(NJC):
                    n_kc = min(NKC, (qb + 1) * (TQB // P))
                    i0, i1 = hp * 2, hp * 2 + 1
                    # h-even: psum rows 0-63 = y, row 64 = denominators
                    yt_a = pyt.tile([P, TQB], F32, tag="yta")
                    # h-odd: psum rows 64-127 = y, row 0 = denominators
                    yt_b = pyt.tile([P, TQB], F32, tag="ytb")
                    acc = pacc.tile([P, TQB], BF16, tag="acc")
                    for kc in range(n_kc):
                        # scores for both heads of the pair -> one 2-bank tile
                        sc = pmm.tile([P, 2 * TQB], F32, tag="mm")
                        for hl in range(2):
                            nc.tensor.matmul(
                                sc[:, hl * TQB:(hl + 1) * TQB],
                                lhsT=t_krot[hp][
                                    hl * hd:(hl + 1) * hd,
                                    kc * P:(kc + 1) * P],
                                rhs=t_qrot[hp][
                                    hl * hd:(hl + 1) * hd,
                                    qb * TQB:(qb + 1) * TQB],
                            )
                        # exp with 1/sqrt(hd) folded in; diag-trim left cols
                        s0 = max(0, kc * P - qb * TQB)
                        ex = pexp.tile([P, 2 * TQB], BF16, tag="exp")
                        sc3 = sc[:].rearrange("p (h w) -> p h w", h=2)
                        ex3 = ex[:].rearrange("p (h w) -> p h w", h=2)
                        if s0 > 0:
                            nc.gpsimd.memset(ex3[:, :, 0:s0], 0.0)
                        nc.scalar.activation(
                            ex3[:, :, s0:TQB],
                            sc3[:, :, s0:TQB],
                            mybir.ActivationFunctionType.Exp,
                            scale=scale,
                        )
                        # diagonal 128-wide triangle mask (tk<=tq kept)
                        if kc * P >= qb * TQB:
                            tri3 = bass.AP(
                                t_tri.tensor, t_tri[:].offset,
                                [t_tri[:].ap[0], [0, 2], t_tri[:].ap[1]],
                            )
                            nc.gpsimd.tensor_mul(
                                ex3[:, :, s0:s0 + P],
                                ex3[:, :, s0:s0 + P],
                                tri3,
                            )
                        # h-odd denominator accumulate on VectorE
                        if kc == 0:
                            nc.vector.tensor_copy(acc[:], ex[:, TQB:2 * TQB])
                        else:
                            nc.vector.tensor_add(
                                acc[:], acc[:], ex[:, TQB:2 * TQB])
                        # P @ V (h-even carries the ones column -> row 64)
                        nc.tensor.matmul(
                            yt_a[0:hd + 1, :],
                            lhsT=t_v[kc][:, i0 * (hd + 1):i0 * (hd + 1) + hd + 1],
                            rhs=ex[:, 0:TQB],
                            start=(kc == 0),
                            stop=(kc == n_kc - 1),
                            skip_group_check=True,
                        )
                        nc.tensor.matmul(
                            yt_b[hd:2 * hd, :],
                            lhsT=t_v[kc][:, i1 * (hd + 1):i1 * (hd + 1) + hd],
                            rhs=ex[:, TQB:2 * TQB],
                            start=(kc == 0),
                            stop=(kc == n_kc - 1),
                            skip_group_check=True,
                        )
                    # h-odd denominators: reduce over partitions into yt_b row 0
                    nc.tensor.matmul(
                        yt_b[0:1, :], lhsT=t_ones[:], rhs=acc[:],
                        skip_group_check=True,
                    )
                    # stage both denominator rows, broadcast via EA matmul,
                    # then 1/s = exp(-ln(s)) on ScalarE
                    with nc.allow_low_precision(reason="bf16 softmax denom"):
                        nc.vector.tensor_copy(
                            t_scp[0:1, :], yt_a[hd:hd + 1, :])
                        nc.vector.tensor_copy(
                            t_scp[hd:hd + 1, :], yt_b[0:1, :])
                    bc = pmm.tile([P, 2 * TQB], F32, tag="mm", name="bc")
                    nc.tensor.matmul(
                        bc[:, 0:TQB], lhsT=t_ea[:], rhs=t_scp[:])
                    rcpb = prcp.tile([P, TQB], F32, tag="rcpb")
                    nc.scalar.activation(
                        rcpb[:], bc[:, 0:TQB],
                        mybir.ActivationFunctionType.Ln)
                    nc.scalar.activation(
                        rcpb[:], rcpb[:], mybir.ActivationFunctionType.Exp,
                        scale=-1.0)
                    nc.vector.tensor_mul(
                        t_yn[hp][0:hd, qb * TQB:(qb + 1) * TQB],
                        yt_a[0:hd, :], rcpb[0:hd, :])
                    nc.vector.tensor_mul(
                        t_yn[hp][hd:2 * hd, qb * TQB:(qb + 1) * TQB],
                        yt_b[hd:2 * hd, :], rcpb[hd:2 * hd, :])

                # ---- output projection for this qb ----
                for m in range(TQB // P):
                    tt = qb * (TQB // P) + m
                    for co in range(C // TQB):
                        zps = pmm.tile([P, 2 * TQB], F32, tag="mm")
                        for jc in range(NJC):
                            nc.tensor.matmul(
                                zps[:, 0:TQB],
                                lhsT=t_yn[jc][:, tt * P:(tt + 1) * P],
                                rhs=t_wp[jc][:, co * TQB:(co + 1) * TQB],
                                start=(jc == 0),
                                stop=(jc == NJC - 1),
                            )
                        zev = pf32.tile([P, TQB], F32, tag="zev", bufs=3)
                        nc.scalar.activation(
                            zev[:], zps[:, 0:TQB],
                            mybir.ActivationFunctionType.Copy)
                        nc.gpsimd.dma_start(
                            z[tt * P:(tt + 1) * P, co * TQB:(co + 1) * TQB],
                            zev[:],
                        )

_ROPE_PERM = np.concatenate([np.arange(0, 64, 2), np.arange(1, 64, 2)])


def _host_inputs(x_b, Wq, bq, Wk, bk, Wv, bv, Wp, heads, T, C, hd):
    """Build the per-core DRAM input dict (numpy)."""
    HD = len(heads) * hd
    rows = np.concatenate([h * hd + _ROPE_PERM for h in heads])
    rows_nop = np.concatenate([np.arange(h * hd, (h + 1) * hd) for h in heads])

    xT = np.ascontiguousarray(x_b.T).astype(NPBF16)
    wqT = np.ascontiguousarray(Wq[rows].T).astype(NPBF16)
    wkT = np.ascontiguousarray(Wk[rows].T).astype(NPBF16)
    wvT = np.ascontiguousarray(Wv[rows_nop].T).astype(NPBF16)
    wpT = np.ascontiguousarray(Wp[:, rows_nop].T).astype(NPBF16)

    j = np.arange(hd // 2, dtype=np.float64)
    inv_freq = 1.0 / (10000.0 ** (2.0 * j / hd))
    t = np.arange(T, dtype=np.float64)
    ang = t[:, None] * inv_freq[None, :]          # [T, 32]
    cos = np.cos(ang)
    sin = np.sin(ang)
    r = np.arange(P)
    cosq = cos[:, r % (hd // 2)].T.astype(np.float32)
    sgn = np.where((r % hd) < hd // 2, -1.0, 1.0)
    sinsq = (sin[:, r % (hd // 2)] * sgn[None, :]).T.astype(np.float32)
    cosq = np.ascontiguousarray(cosq)
    sinsq = np.ascontiguousarray(sinsq)

    pair = np.where((r % hd) < hd // 2, r + hd // 2, r - hd // 2)
    jmat = np.zeros((P, P), np.float32)
    jmat[pair, r] = 1.0
    tri = (np.arange(P)[None, :] >= np.arange(P)[:, None]).astype(np.float32)
    ea = np.zeros((P, P), np.float32)
    ea[(r // hd) * hd, r] = 1.0

    bqTh = bq[rows].reshape(HD // P, P).T.astype(np.float32)
    bkTh = bk[rows].reshape(HD // P, P).T.astype(np.float32)
    bvb = np.tile(bv[rows_nop][None, :], (P, 1)).astype(np.float32)

    return {
        "xT": xT, "wqT": wqT, "wkT": wkT, "wvT": wvT, "wpT": wpT,
        "cosq": cosq, "sinsq": sinsq,
        "jmat": jmat.astype(NPBF16), "tri": tri.astype(NPBF16),
        "ea": ea.astype(NPBF16),
        "onesc": np.ones((P, 1), NPBF16),
        "bqT": np.ascontiguousarray(bqTh),
        "bkT": np.ascontiguousarray(bkTh),
        "bvb": bvb,
    }


def make_core_inputs(x, Wq, bq, Wk, bk, Wv, bv, Wp, T=2048, C=1024, hd=64,
                     heads_per_core=4):
    in_maps = []
    for c in range(N_CORES):
        b = c // 4
        g = c % 4
        heads = list(range(g * heads_per_core, (g + 1) * heads_per_core))
        in_maps.append(_host_inputs(
            np.asarray(x[b]), Wq, bq, Wk, bk, Wv, bv, Wp, heads, T, C, hd))
    return in_maps


def kernel(x, Wq, bq, Wk, bk, Wv, bv, Wp, bp):
    x = np.asarray(x, np.float32)
    Wq = np.asarray(Wq, np.float32)
    bq = np.asarray(bq, np.float32)
    Wk = np.asarray(Wk, np.float32)
    bk = np.asarray(bk, np.float32)
    Wv = np.asarray(Wv, np.float32)
    bv = np.asarray(bv, np.float32)
    Wp = np.asarray(Wp, np.float32)
    bp = np.asarray(bp, np.float32)
    B, T, C = x.shape

    _patch_act_tables()
    nc = bacc.Bacc("TRN2", target_bir_lowering=False, debug=False,
                   num_devices=N_CORES)
    build_attention_kernel(nc, T=T, C=C)
    nc.compile()

    in_maps = make_core_inputs(x, Wq, bq, Wk, bk, Wv, bv, Wp, T=T, C=C)
    res = run_bass_kernel_spmd(nc, in_maps, list(range(N_CORES)))

    out = np.zeros((B, T, C), np.float32)
    for c in range(N_CORES):
        out[c // 4] += res.results[c]["z"]
    out += bp[None, None, :]
    return out


if __name__ == "__main__":
    import reference

    inputs = reference.setup_inputs()
    expected = np.asarray(reference.reference(**inputs))
    actual = kernel(**{k: np.asarray(v) for k, v in inputs.items()})
    err = np.abs(actual - expected).max() / np.abs(expected).max()
    print("Relative error:", err)



# revision 4
# speedup vs baseline: 1.5329x; 1.5329x over previous
"""Causal self-attention (B=2, T=2048, C=1024, H=16, RoPE) on 8 TRN2 cores.

Sharding: data-parallel over B (2 groups of 4 cores) x tensor-parallel over
heads (4 heads per core). Each core computes q/k/v projections for its heads,
RoPE, causal attention, and its partial output projection; the host sums the
4 partial projections per batch and adds bp.

Schedule: projections for T-block tb+1 and the output projection for block
qb-1 are emitted interleaved with the attention chunk loop of block qb, so
TensorE matmul work fills the gaps while ScalarE works through the softmax
exps (the Tile scheduler pops ready work by emission priority). Input DMAs
are batched into a handful of multi-part descriptors issued on the idle
Sync engine so compute starts ~4us in instead of after a bulk load.

Layout choices (per core):
  - x resident in SBUF as one [128, 8*2048] tile (contraction chunks side
    by side); q, k produced TRANSPOSED: qT/kT [256=4heads*64, T] via
    lhsT=W, rhs=xT. Head-dim pairs are pre-permuted (evens|odds) in the
    weights so RoPE needs no strided access; the pair-swap is a constant
    permutation matmul (J), combined on VectorE in bf16.
  - v produced NON-transposed [T, 4 heads]; per head pair the SBUF layout
    is [v_even(64) | 1 | 1 | 0*63 | v_odd(64)] so BOTH heads' PV matmuls
    also emit their softmax denominator rows (even: psum row 64, odd: psum
    row 0) with no extra reduction work.
  - scores computed transposed: ST[tk, tq] = k_rot @ q_rot^T per head
    (two heads row-packed into PE quadrants), softmax-exp elementwise on
    ScalarE (scale folded), causal handling = per-chunk left-trim of the
    matmul/exp range + a fixed 128x128 triangle multiply on GpSimd.
  - denominators: staged bf16, broadcast over partitions by a constant
    matmul (EA), reciprocal via the fast approximate DVE op.
"""

import math

import numpy as np
import ml_dtypes

import concourse.bass as bass
import concourse.bacc as bacc
import concourse.mybir as mybir
from concourse.tile import TileContext
from concourse.bass_utils import run_bass_kernel_spmd

BF16 = mybir.dt.bfloat16
F32 = mybir.dt.float32
NPBF16 = ml_dtypes.bfloat16

N_CORES = 8
P = 128

_UNIFIED_ACT_SET = "natural_log_exp_and_others"


def _patch_act_tables():
    import concourse.hw_specs as _hw
    import concourse.bacc as _bacc
    if getattr(_bacc, "_act_tables_patched", False):
        return
    _orig = _hw.get_activation_tables

    def _gat(arch):
        tabs = _orig(arch)
        if _UNIFIED_ACT_SET in tabs:
            keep = tabs[_UNIFIED_ACT_SET]
            drop = {
                mybir.ActivationFunctionType.Exp,
                mybir.ActivationFunctionType.Copy,
            } & keep
            for name, fns in tabs.items():
                if name != _UNIFIED_ACT_SET:
                    for f in drop:
                        fns.discard(f)
        return tabs

    _bacc.get_activation_tables = _gat
    _bacc._act_tables_patched = True


def build_attention_kernel(nc, T=2048, C=1024, n_heads=4, hd=64):
    """Emit the per-core kernel. Returns nothing; tensors are declared on nc."""
    _patch_act_tables()
    HD = n_heads * hd            # 256: local head dims
    KC = C // P                  # 8: contraction chunks for projections
    NJC = HD // P                # 2: partition tiles of qT/kT (head pairs)
    TQB = 512                    # tq block for scores/PV
    NQB = T // TQB               # 4
    NTT = T // P                 # 16: t tiles for v
    VW = 2 * hd + 65             # 193: per-pair v columns
    scale = 1.0 / math.sqrt(hd)

    # ---- DRAM I/O ----
    xT = nc.declare_dram_parameter("xT", [C, T], BF16, isOutput=False)
    wqT = nc.declare_dram_parameter("wqT", [C, HD], BF16, isOutput=False)
    wkT = nc.declare_dram_parameter("wkT", [C, HD], BF16, isOutput=False)
    wvT = nc.declare_dram_parameter("wvT", [C, HD], BF16, isOutput=False)
    wpT = nc.declare_dram_parameter("wpT", [HD, C], BF16, isOutput=False)
    cosq = nc.declare_dram_parameter("cosq", [P, T], BF16, isOutput=False)
    sinsq = nc.declare_dram_parameter("sinsq", [P, T], BF16, isOutput=False)
    jmat = nc.declare_dram_parameter("jmat", [P, P], BF16, isOutput=False)
    tri = nc.declare_dram_parameter("tri", [P, P], BF16, isOutput=False)
    ea = nc.declare_dram_parameter("ea", [P, P], BF16, isOutput=False)
    bqT = nc.declare_dram_parameter("bqT", [P, NJC], F32, isOutput=False)
    bkT = nc.declare_dram_parameter("bkT", [P, NJC], F32, isOutput=False)
    bvb = nc.declare_dram_parameter("bvb", [P, HD], F32, isOutput=False)
    z = nc.declare_dram_parameter("z", [T, C], BF16, isOutput=True)

    with TileContext(nc) as tc:
        import contextlib

        with contextlib.ExitStack() as ctx:
            # ---- persistent SBUF pools ----
            pc = ctx.enter_context(tc.tile_pool(name="const", bufs=1))
            px = ctx.enter_context(tc.tile_pool(name="x", bufs=1))
            pw = ctx.enter_context(tc.tile_pool(name="w", bufs=1))
            pqk = ctx.enter_context(tc.tile_pool(name="qk", bufs=1))
            pv = ctx.enter_context(tc.tile_pool(name="v", bufs=1))
            py = ctx.enter_context(tc.tile_pool(name="y", bufs=1))
            # transient SBUF pools
            praw = ctx.enter_context(tc.tile_pool(name="raw", bufs=4))
            pjq = ctx.enter_context(tc.tile_pool(name="jq", bufs=4))
            prt = ctx.enter_context(tc.tile_pool(name="ropetmp", bufs=6))
            pexp = ctx.enter_context(tc.tile_pool(name="exp", bufs=8))
            prcp = ctx.enter_context(tc.tile_pool(name="rcp", bufs=2))
            pzev = ctx.enter_context(tc.tile_pool(name="zev", bufs=3))
            # PSUM pools: 2*2 + 2*1 + 2*1 = 8 banks
            pmm = ctx.enter_context(
                tc.tile_pool(name="mm", bufs=2, space="PSUM"))
            pyt = ctx.enter_context(
                tc.tile_pool(name="yt", bufs=1, space="PSUM"))
            pps = ctx.enter_context(
                tc.tile_pool(name="pp", bufs=2, space="PSUM"))

            # ---- batched input DMAs on the (idle) Sync engine ----
            t_j = pc.tile([P, P], BF16, tag="j")
            nc.sync.dma_start(t_j[:], jmat[:])
            t_tri = pc.tile([P, P], BF16, tag="tri")
            nc.sync.dma_start(t_tri[:], tri[:])
            t_ea = pc.tile([P, P], BF16, tag="ea")
            nc.sync.dma_start(t_ea[:], ea[:])
            t_bq = pc.tile([P, NJC], F32, tag="bq")
            nc.sync.dma_start(t_bq[:], bqT[:])
            t_bk = pc.tile([P, NJC], F32, tag="bk")
            nc.sync.dma_start(t_bk[:], bkT[:])
            t_bv = pc.tile([P, HD], F32, tag="bv")
            nc.sync.dma_start(t_bv[:], bvb[:])

            # weights: one multi-part DMA per tensor
            t_wv = pw.tile([P, KC * HD], BF16, tag="wv")
            nc.sync.dma_start(
                t_wv[:].rearrange("p (k c) -> p k c", k=KC),
                wvT[:].rearrange("(k p) c -> p k c", p=P))
            t_wq = pw.tile([P, KC * HD], BF16, tag="wq")
            nc.sync.dma_start(
                t_wq[:].rearrange("p (k c) -> p k c", k=KC),
                wqT[:].rearrange("(k p) c -> p k c", p=P))
            t_wk = pw.tile([P, KC * HD], BF16, tag="wk")
            nc.sync.dma_start(
                t_wk[:].rearrange("p (k c) -> p k c", k=KC),
                wkT[:].rearrange("(k p) c -> p k c", p=P))
            t_cos = pc.tile([P, T], BF16, tag="cos")
            nc.sync.dma_start(t_cos[:], cosq[:])
            t_sin = pc.tile([P, T], BF16, tag="sin")
            nc.sync.dma_start(t_sin[:], sinsq[:])

            # x: one [128, KC*T] tile, loaded in 4 T-block parts so early
            # projection blocks can start while the rest streams in
            t_x = px.tile([P, KC * T], BF16, tag="x")
            x3d = t_x[:].rearrange("p (k t) -> p k t", k=KC)
            xs3d = xT[:].rearrange("(k p) t -> p k t", p=P)
            for tb in range(NQB):
                sl = slice(tb * TQB, (tb + 1) * TQB)
                nc.sync.dma_start(x3d[:, :, sl], xs3d[:, :, sl])

            t_wp = pw.tile([P, NJC * C], BF16, tag="wp")
            nc.sync.dma_start(
                t_wp[:].rearrange("p (j c) -> p j c", j=NJC),
                wpT[:].rearrange("(j p) c -> p j c", p=P))

            # denominator staging tiles (rows 0/64 carry data, rest stay 1.0)
            t_scp = []
            for i in range(2):
                s = pc.tile([P, TQB], BF16, tag=f"scp{i}", name=f"scp{i}")
                nc.vector.memset(s[:], 1.0)
                t_scp.append(s)

            # persistent targets
            t_qrot = [pqk.tile([P, T], BF16, tag=f"qr{jc}", name=f"qrot{jc}")
                      for jc in range(NJC)]
            t_krot = [pqk.tile([P, T], BF16, tag=f"kr{jc}", name=f"krot{jc}")
                      for jc in range(NJC)]
            t_yn = [py.tile([P, T], BF16, tag=f"yn{jc}", name=f"yn{jc}")
                    for jc in range(NJC)]
            t_v = [pv.tile([P, NJC * VW], BF16, tag=f"v{tt}", name=f"v{tt}")
                   for tt in range(NTT)]

            def wsl(w, k, jc):
                return w[:, k * HD + jc * P:k * HD + (jc + 1) * P]

            def xsl(k, lo, hi):
                return t_x[:, k * T + lo:k * T + hi]

            # ---- emission quanta ----
            def q_vproj(tt):
                def go():
                    vps = pps.tile([P, TQB], F32, tag="pp")
                    for k in range(KC):
                        nc.tensor.matmul(
                            vps[:, 0:HD],
                            lhsT=xsl(k, tt * P, (tt + 1) * P),
                            rhs=t_wv[:, k * HD:(k + 1) * HD],
                            start=(k == 0),
                            stop=(k == KC - 1),
                        )
                    vt = t_v[tt]
                    # heads p*2 (even) at cols [p*VW, p*VW+64)
                    dst_e = bass.AP(
                        vt.tensor, vt[:].offset,
                        [vt[:].ap[0], [VW, NJC], [1, hd]])
                    src_e = bass.AP(
                        vps.tensor, vps[:].offset,
                        [vps[:].ap[0], [2 * hd, NJC], [1, hd]])
                    b_e = bass.AP(
                        t_bv.tensor, t_bv[:].offset,
                        [t_bv[:].ap[0], [2 * hd, NJC], [1, hd]])
                    nc.vector.tensor_add(dst_e, src_e, b_e)
                    # heads p*2+1 (odd) at cols [p*VW+129, p*VW+193)
                    dst_o = bass.AP(
                        vt.tensor, vt[:].offset + (2 * hd + 1),
                        [vt[:].ap[0], [VW, NJC], [1, hd]])
                    src_o = bass.AP(
                        vps.tensor, vps[:].offset + hd,
                        [vps[:].ap[0], [2 * hd, NJC], [1, hd]])
                    b_o = bass.AP(
                        t_bv.tensor, t_bv[:].offset + hd,
                        [t_bv[:].ap[0], [2 * hd, NJC], [1, hd]])
                    nc.vector.tensor_add(dst_o, src_o, b_o)
                    # ones at cols {64, 65}, zeros at [66, 129) per pair
                    ones_ap = bass.AP(
                        vt.tensor, vt[:].offset + hd,
                        [vt[:].ap[0], [VW, NJC], [1, 2]])
                    nc.gpsimd.memset(ones_ap, 1.0)
                    zer_ap = bass.AP(
                        vt.tensor, vt[:].offset + hd + 2,
                        [vt[:].ap[0], [VW, NJC], [1, hd - 1]])
                    nc.gpsimd.memset(zer_ap, 0.0)
                return go

            def q_qkproj(dst, wten, bias, jc, tb):
                def go():
                    lo, hi = tb * TQB, (tb + 1) * TQB
                    qps = pps.tile([P, TQB], F32, tag="pp")
                    for k in range(KC):
                        nc.tensor.matmul(
                            qps[:],
                            lhsT=wsl(wten, k, jc),
                            rhs=xsl(k, lo, hi),
                            start=(k == 0),
                            stop=(k == KC - 1),
                        )
                    raw = praw.tile([P, TQB], BF16, tag="raw")
                    nc.vector.tensor_scalar_add(
                        raw[:], qps[:], bias[:, jc:jc + 1])
                    jps = pps.tile([P, TQB], F32, tag="pp")
                    nc.tensor.matmul(jps[:], lhsT=t_j[:], rhs=raw[:])
                    jq = pjq.tile([P, TQB], BF16, tag="jq")
                    with nc.allow_low_precision(reason="bf16 rope"):
                        nc.any.tensor_copy(jq[:], jps[:])
                        tc1 = prt.tile([P, TQB], BF16, tag="rt")
                        nc.vector.tensor_mul(tc1[:], raw[:], t_cos[:, lo:hi])
                        tc2 = prt.tile([P, TQB], BF16, tag="rt")
                        nc.vector.tensor_mul(tc2[:], jq[:], t_sin[:, lo:hi])
                        nc.vector.tensor_add(dst[:, lo:hi], tc1[:], tc2[:])
                return go

            def q_chunk(qb, hp, kc, n_kc, yt_a, yt_b):
                def go():
                    s0 = max(0, kc * P - qb * TQB)
                    sc = pmm.tile([P, 2 * TQB], F32, tag="sc")
                    for hl in range(2):
                        nc.tensor.matmul(
                            sc[:, hl * TQB + s0:(hl + 1) * TQB],
                            lhsT=t_krot[hp][
                                hl * hd:(hl + 1) * hd,
                                kc * P:(kc + 1) * P],
                            rhs=t_qrot[hp][
                                hl * hd:(hl + 1) * hd,
                                qb * TQB + s0:(qb + 1) * TQB],
                        )
                    ex = pexp.tile([P, 2 * TQB], BF16, tag="exp")
                    sc3 = sc[:].rearrange("p (h w) -> p h w", h=2)
                    ex3 = ex[:].rearrange("p (h w) -> p h w", h=2)
                    nc.scalar.activation(
                        ex3[:, :, s0:TQB],
                        sc3[:, :, s0:TQB],
                        mybir.ActivationFunctionType.Exp,
                        scale=scale,
                    )
                    # diagonal 128-wide triangle mask (tk<=tq kept)
                    if kc * P >= qb * TQB:
                        tri3 = bass.AP(
                            t_tri.tensor, t_tri[:].offset,
                            [t_tri[:].ap[0], [0, 2], t_tri[:].ap[1]],
                        )
                        nc.gpsimd.tensor_mul(
                            ex3[:, :, s0:s0 + P],
                            ex3[:, :, s0:s0 + P],
                            tri3,
                        )
                    # P @ V; both heads' denominators ride along:
                    # even head -> yt_a row 64, odd head -> yt_b row 0
                    vt = t_v[kc]
                    nc.tensor.matmul(
                        yt_a[0:hd + 1, s0:TQB],
                        lhsT=vt[:, hp * VW:hp * VW + hd + 1],
                        rhs=ex[:, s0:TQB],
                        start=(kc == 0),
                        stop=(kc == n_kc - 1),
                        skip_group_check=True,
                    )
                    nc.tensor.matmul(
                        yt_b[:, s0:TQB],
                        lhsT=vt[:, hp * VW + hd + 1:(hp + 1) * VW],
                        rhs=ex[:, TQB + s0:2 * TQB],
                        start=(kc == 0),
                        stop=(kc == n_kc - 1),
                        skip_group_check=True,
                    )
                return go

            def q_tail(qb, hp, yt_a, yt_b):
                def go():
                    scp = t_scp[(qb * NJC + hp) % 2]
                    with nc.allow_low_precision(reason="bf16 softmax denom"):
                        nc.vector.tensor_copy(
                            scp[0:1, :], yt_a[hd:hd + 1, :])
                        nc.vector.tensor_copy(
                            scp[hd:hd + 1, :], yt_b[0:1, :])
                    bc = pps.tile([P, TQB], F32, tag="pp", name="bc")
                    nc.tensor.matmul(bc[:], lhsT=t_ea[:], rhs=scp[:])
                    rcp = prcp.tile([P, TQB], F32, tag="rcpb")
                    nc.vector.reciprocal_approx_fast(rcp[:], bc[:])
                    with nc.allow_low_precision(reason="bf16 y"):
                        nc.vector.tensor_mul(
                            t_yn[hp][0:hd, qb * TQB:(qb + 1) * TQB],
                            yt_a[0:hd, :], rcp[0:hd, :])
                        nc.vector.tensor_mul(
                            t_yn[hp][hd:2 * hd, qb * TQB:(qb + 1) * TQB],
                            yt_b[hd:2 * hd, :], rcp[hd:2 * hd, :])
                return go

            def q_outproj(tt):
                def go():
                    zev = pzev.tile([P, C], BF16, tag="zev")
                    for co in range(C // TQB):
                        zps = pps.tile([P, TQB], F32, tag="pp")
                        for jc in range(NJC):
                            nc.tensor.matmul(
                                zps[:],
                                lhsT=t_yn[jc][:, tt * P:(tt + 1) * P],
                                rhs=t_wp[:, jc * C + co * TQB:
                                         jc * C + (co + 1) * TQB],
                                start=(jc == 0),
                                stop=(jc == NJC - 1),
                            )
                        with nc.allow_low_precision(reason="bf16 z"):
                            nc.any.tensor_copy(
                                zev[:, co * TQB:(co + 1) * TQB], zps[:])
                    nc.sync.dma_start(
                        z[tt * P:(tt + 1) * P, :], zev[:])
                return go

            def proj_quanta(tb):
                qs = [q_vproj(tt) for tt in range(4 * tb, 4 * tb + 4)]
                for jc in range(NJC):
                    qs.append(q_qkproj(t_qrot[jc], t_wq, t_bq, jc, tb))
                    qs.append(q_qkproj(t_krot[jc], t_wk, t_bk, jc, tb))
                return qs

            def attn_quanta(qb):
                qs = []
                n_kc = min(T // P, (qb + 1) * (TQB // P))
                for hp in range(NJC):
                    yt_a = pyt.tile([P, TQB], F32, tag="yta")
                    yt_b = pyt.tile([P, TQB], F32, tag="ytb")
                    for kc in range(n_kc):
                        qs.append(q_chunk(qb, hp, kc, n_kc, yt_a, yt_b))
                    qs.append(q_tail(qb, hp, yt_a, yt_b))
                return qs

            def merge(primary, fillers):
                if not primary:
                    for f in fillers:
                        f()
                    return
                ratio = len(fillers) / len(primary)
                acc = 0.0
                fi = 0
                for p in primary:
                    p()
                    acc += ratio
                    while acc >= 1.0 and fi < len(fillers):
                        fillers[fi]()
                        fi += 1
                        acc -= 1.0
                while fi < len(fillers):
                    fillers[fi]()
                    fi += 1

            # ---- schedule ----
            for q in proj_quanta(0):
                q()
            for qb in range(NQB):
                fillers = []
                if qb + 1 < NQB:
                    fillers += proj_quanta(qb + 1)
                if qb > 0:
                    fillers += [q_outproj(tt)
                                for tt in range(4 * (qb - 1), 4 * qb)]
                merge(attn_quanta(qb), fillers)
            for tt in range(4 * (NQB - 1), 4 * NQB):
                q_outproj(tt)()


_ROPE_PERM = np.concatenate([np.arange(0, 64, 2), np.arange(1, 64, 2)])


def _host_inputs(x_b, Wq, bq, Wk, bk, Wv, bv, Wp, heads, T, C, hd):
    """Build the per-core DRAM input dict (numpy)."""
    HD = len(heads) * hd
    rows = np.concatenate([h * hd + _ROPE_PERM for h in heads])
    rows_nop = np.concatenate([np.arange(h * hd, (h + 1) * hd) for h in heads])

    xT = np.ascontiguousarray(x_b.T).astype(NPBF16)
    wqT = np.ascontiguousarray(Wq[rows].T).astype(NPBF16)
    wkT = np.ascontiguousarray(Wk[rows].T).astype(NPBF16)
    wvT = np.ascontiguousarray(Wv[rows_nop].T).astype(NPBF16)
    wpT = np.ascontiguousarray(Wp[:, rows_nop].T).astype(NPBF16)

    j = np.arange(hd // 2, dtype=np.float64)
    inv_freq = 1.0 / (10000.0 ** (2.0 * j / hd))
    t = np.arange(T, dtype=np.float64)
    ang = t[:, None] * inv_freq[None, :]          # [T, 32]
    cos = np.cos(ang)
    sin = np.sin(ang)
    r = np.arange(P)
    cosq = cos[:, r % (hd // 2)].T.astype(np.float32)
    sgn = np.where((r % hd) < hd // 2, -1.0, 1.0)
    sinsq = (sin[:, r % (hd // 2)] * sgn[None, :]).T.astype(np.float32)

    pair = np.where((r % hd) < hd // 2, r + hd // 2, r - hd // 2)
    jmat = np.zeros((P, P), np.float32)
    jmat[pair, r] = 1.0
    tri = (np.arange(P)[None, :] >= np.arange(P)[:, None]).astype(np.float32)
    ea = np.zeros((P, P), np.float32)
    ea[(r // hd) * hd, r] = 1.0

    bqTh = bq[rows].reshape(HD // P, P).T.astype(np.float32)
    bkTh = bk[rows].reshape(HD // P, P).T.astype(np.float32)
    bvb = np.tile(bv[rows_nop][None, :], (P, 1)).astype(np.float32)

    return {
        "xT": xT, "wqT": wqT, "wkT": wkT, "wvT": wvT, "wpT": wpT,
        "cosq": np.ascontiguousarray(cosq).astype(NPBF16),
        "sinsq": np.ascontiguousarray(sinsq).astype(NPBF16),
        "jmat": jmat.astype(NPBF16), "tri": tri.astype(NPBF16),
        "ea": ea.astype(NPBF16),
        "bqT": np.ascontiguousarray(bqTh),
        "bkT": np.ascontiguousarray(bkTh),
        "bvb": bvb,
    }


def make_core_inputs(x, Wq, bq, Wk, bk, Wv, bv, Wp, T=2048, C=1024, hd=64,
                     heads_per_core=4):
    in_maps = []
    for c in range(N_CORES):
        b = c // 4
        g = c % 4
        heads = list(range(g * heads_per_core, (g + 1) * heads_per_core))
        in_maps.append(_host_inputs(
            np.asarray(x[b]), Wq, bq, Wk, bk, Wv, bv, Wp, heads, T, C, hd))
    return in_maps


def kernel(x, Wq, bq, Wk, bk, Wv, bv, Wp, bp):
    x = np.asarray(x, np.float32)
    Wq = np.asarray(Wq, np.float32)
    bq = np.asarray(bq, np.float32)
    Wk = np.asarray(Wk, np.float32)
    bk = np.asarray(bk, np.float32)
    Wv = np.asarray(Wv, np.float32)
    bv = np.asarray(bv, np.float32)
    Wp = np.asarray(Wp, np.float32)
    bp = np.asarray(bp, np.float32)
    B, T, C = x.shape

    _patch_act_tables()
    nc = bacc.Bacc("TRN2", target_bir_lowering=False, debug=False,
                   num_devices=N_CORES)
    build_attention_kernel(nc, T=T, C=C)
    nc.compile()

    in_maps = make_core_inputs(x, Wq, bq, Wk, bk, Wv, bv, Wp, T=T, C=C)
    res = run_bass_kernel_spmd(nc, in_maps, list(range(N_CORES)))

    out = np.zeros((B, T, C), np.float32)
    for c in range(N_CORES):
        out[c // 4] += res.results[c]["z"].astype(np.float32)
    out += bp[None, None, :]
    return out


if __name__ == "__main__":
    import reference

    inputs = reference.setup_inputs()
    expected = np.asarray(reference.reference(**inputs))
    actual = kernel(**{k: np.asarray(v) for k, v in inputs.items()})
    err = np.abs(actual - expected).max() / np.abs(expected).max()
    print("Relative error:", err)
